# revision 13
# baseline (speedup 1.0000x reference)
"""Trainium2 Bass kernel for BlankEmbedding (embedding lookup + blank shift-accumulate).

Reference semantics:
    out = emb[x]                               # [B, S, D]
    preblank[s] = (x[s+1]==BLANK) & (x[s]!=BLANK)   (per row; preblank[S-1]=0)
    out[s] += sum_{k=1..3} preblank[s-k] * emb[x[s-k]]   (zero-pad at row start)

Strategy: data-parallel over the 16384 flattened tokens, 2048 per core.
Each core holds the full table in DRAM and gathers with per-partition-
index indirect DMAs (SWDGE indirect DMA is limited to 128 indices per
instruction on HW and costs ~1.5us of GPSIMD descriptor-generation
each). Tiles OVERLAP by 3 tokens: tile i's partition q holds token
125*i + q - 3 (3-row halo + 125 fresh tokens), 17 tiles per core, so
every shift source lives in-tile and no cross-tile edge handling is
needed. The first tile's halo covers runs crossing core boundaries
(blank-filled at row starts, which forces the mask to 0 there, matching
the reference zero-padding).

The shift-accumulate runs on the tensor engine in exact fp32 with the
base folded in: psum_i = M_i.T @ g_i where M_i = I + A*w_i and A[q,p]=1
iff 1 <= p-q <= 3 — just two 512-wide matmuls per tile. The preblank
mask w comes from host-pre-laid [128,17] token/next-token columns in
five batched vector ops. The PSUM result is copied to SBUF and stored
from partitions 3..127 (output rows 125*i .. 125*i+124) with plain
HWDGE DMAs; psum rows 0..2 are halo junk and are never stored.
"""

import numpy as np

VOCAB = 50257
DIM = 1024
BLANK = 100
B, S = 4, 4096
N_CORES = 8
TOK = B * S                  # 16384 flattened tokens
TPC = TOK // N_CORES         # 2048 tokens per core
P = 128                      # SBUF partitions
HALO = 3                     # max shift distance
STEP = P - HALO              # 125 fresh tokens per tile
NT = -(-TPC // STEP)         # 17 overlapped tiles per core
EXT = HALO + NT * STEP + 1   # token stream + halo + pad (2129)
NMM = DIM // 512             # matmul free-dim chunks per tile

_CACHE = {}


def _shift_consts():
    """A: shift-accumulate band A[q,p]=1 iff 1<=p-q<=3; I: identity."""
    q = np.arange(P)[:, None]
    p = np.arange(P)[None, :]
    a_mat = ((p - q >= 1) & (p - q <= HALO)).astype(np.float32)
    i_mat = np.eye(P, dtype=np.float32)
    return a_mat, i_mat


def _build_nc():
    from concourse import bacc, mybir, tile
    import concourse.bass as bass

    nc = bacc.Bacc(
        "TRN2", target_bir_lowering=False, debug=False, num_devices=N_CORES
    )
    i32 = mybir.dt.int32
    f32 = mybir.dt.float32

    ix_dram = nc.dram_tensor("ix_cols", [P, NT], i32, kind="ExternalInput")
    ixn_dram = nc.dram_tensor("ixn_cols", [P, NT], i32, kind="ExternalInput")
    emb = nc.dram_tensor("emb", [VOCAB, DIM], f32, kind="ExternalInput")
    a_dram = nc.dram_tensor("a_mat", [P, P], f32, kind="ExternalInput")
    i_dram = nc.dram_tensor("i_mat", [P, P], f32, kind="ExternalInput")
    out = nc.dram_tensor("out", [TPC, DIM], f32, kind="ExternalOutput")

    with tile.TileContext(nc) as tc:
        with (
            tc.tile_pool(name="sbuf", bufs=1) as pool,
            tc.tile_pool(name="psum", bufs=3, space="PSUM") as psum_pool,
        ):
            ix_all = pool.tile([P, NT], i32)
            ixn_all = pool.tile([P, NT], i32)
            nc.scalar.dma_start(out=ix_all[:], in_=ix_dram[:])
            nc.scalar.dma_start(out=ixn_all[:], in_=ixn_dram[:])
            a_sb = pool.tile([P, P], f32)
            i_sb = pool.tile([P, P], f32)
            nc.scalar.dma_start(out=a_sb[:], in_=a_dram[:])
            nc.scalar.dma_start(out=i_sb[:], in_=i_dram[:])

            # ---- preblank masks w = isblank(next) & ~isblank(cur), batched ----
            b_all = pool.tile([P, NT], i32)
            bn_all = pool.tile([P, NT], i32)
            w_all = pool.tile([P, NT], f32)
            nc.vector.tensor_scalar(
                out=b_all[:], in0=ix_all[:], scalar1=BLANK, scalar2=None,
                op0=mybir.AluOpType.is_equal,
            )
            nc.vector.tensor_scalar(
                out=bn_all[:], in0=ixn_all[:], scalar1=BLANK, scalar2=None,
                op0=mybir.AluOpType.is_equal,
            )
            nc.vector.tensor_scalar(  # b := 1 - b
                out=b_all[:], in0=b_all[:], scalar1=-1, scalar2=1,
                op0=mybir.AluOpType.mult, op1=mybir.AluOpType.add,
            )
            nc.vector.tensor_tensor(  # bn := bn * (1 - b)
                out=bn_all[:], in0=bn_all[:], in1=b_all[:],
                op=mybir.AluOpType.mult,
            )
            nc.vector.tensor_copy(out=w_all[:], in_=bn_all[:])

            # ---- per-tile gather / matmul / copy / store ----
            for i in range(NT):
                gt = pool.tile([P, DIM], f32, name=f"g{i}", tag="g", bufs=6)
                nc.gpsimd.indirect_dma_start(
                    out=gt[:], out_offset=None, in_=emb[:],
                    in_offset=bass.IndirectOffsetOnAxis(
                        ap=ix_all[:, i : i + 1], axis=0
                    ),
                )
                m_sb = pool.tile([P, P], f32, name=f"m{i}", tag="m", bufs=4)
                nc.vector.tensor_tensor(  # M = A * w_i (bcast)
                    out=m_sb[:], in0=a_sb[:],
                    in1=w_all[:, i : i + 1].to_broadcast([P, P]),
                    op=mybir.AluOpType.mult,
                )
                nc.vector.tensor_tensor(  # M += I  (fold base into matmul)
                    out=m_sb[:], in0=m_sb[:], in1=i_sb[:],
                    op=mybir.AluOpType.add,
                )
                c = psum_pool.tile([P, DIM], f32, name=f"c{i}", tag="c")
                for h in range(NMM):
                    sl = slice(512 * h, 512 * (h + 1))
                    nc.tensor.matmul(
                        out=c[:, sl], lhsT=m_sb[:], rhs=gt[:, sl],
                        start=True, stop=True,
                    )
                o_sb = pool.tile([P, DIM], f32, name=f"o{i}", tag="o", bufs=4)
                nc.vector.tensor_copy(out=o_sb[:], in_=c[:])
                r0 = STEP * i
                nrow = min(STEP, TPC - r0)
                nc.sync.dma_start(
                    out=out[r0 : r0 + nrow, :],
                    in_=o_sb[HALO : HALO + nrow, :],
                )

    nc.compile()
    return nc


def get_nc():
    if "nc" not in _CACHE:
        _CACHE["nc"] = _build_nc()
    return _CACHE["nc"]


def shard_inputs(x, emb_table):
    """Build per-core in_maps from full inputs."""
    flat = np.ascontiguousarray(np.asarray(x).astype(np.int32).reshape(-1))
    emb_f32 = np.ascontiguousarray(np.asarray(emb_table, dtype=np.float32))
    a_mat, i_mat = _shift_consts()
    in_maps = []
    for c in range(N_CORES):
        start = c * TPC
        ext = np.zeros(EXT, dtype=np.int32)
        if start % S == 0:
            # row start: blank-filled halo makes the preblank mask 0 there,
            # matching the reference's zero-padded shifts at row boundaries
            ext[:HALO] = BLANK
        else:
            ext[:HALO] = flat[start - HALO : start]
        ext[HALO : HALO + TPC] = flat[start : start + TPC]
        # trailing pad stays 0 (non-blank): those positions are never stored
        # and can never become preblanks

        # overlapped tiles: tile i, partition q holds ext[125*i + q]
        ix_cols = np.empty((P, NT), dtype=np.int32)
        ixn_cols = np.empty((P, NT), dtype=np.int32)
        for i in range(NT):
            ix_cols[:, i] = ext[STEP * i : STEP * i + P]
            ixn_cols[:, i] = ext[STEP * i + 1 : STEP * i + P + 1]
        in_maps.append(
            {"ix_cols": ix_cols, "ixn_cols": ixn_cols, "emb": emb_f32,
             "a_mat": a_mat, "i_mat": i_mat}
        )
    return in_maps


def assemble_output(results):
    parts = [results[c]["out"] for c in range(N_CORES)]
    return np.concatenate(parts, axis=0).reshape(B, S, DIM)


def kernel(x, emb_table):
    from concourse.bass_utils import run_bass_kernel_spmd

    nc = get_nc()
    in_maps = shard_inputs(x, emb_table)
    res = run_bass_kernel_spmd(nc, in_maps, core_ids=list(range(N_CORES)))
    return assemble_output(res.results)


# revision 15
# speedup vs baseline: 1.2780x; 1.2780x over previous
"""Trainium2 Bass kernel for BlankEmbedding (embedding lookup + blank shift-accumulate).

Reference semantics:
    out = emb[x]                               # [B, S, D]
    preblank[s] = (x[s+1]==BLANK) & (x[s]!=BLANK)   (per row; preblank[S-1]=0)
    out[s] += sum_{k=1..3} preblank[s-k] * emb[x[s-k]]   (zero-pad at row start)

Strategy: data-parallel over the 16384 flattened tokens, 2048 per core.
Each core holds the full table in DRAM and gathers its 2048 rows with
per-partition-index indirect DMAs (17 instructions: 16 token tiles of
[128, DIM] with token t = 128*i + p, plus one halo tile; SWDGE indirect
DMA is limited to 128 indices per instruction on HW and costs ~1.5us of
GPSIMD descriptor-generation each, so the count is kept minimal).

The shift-accumulate runs on the tensor engine with base folded in:
out_i = M_i.T @ g_i + (E*w_{i-1}).T @ g_{i-1}, where M_i = I + A*w_i,
A[q,p] = 1 iff 1 <= p-q <= 3 (in-tile shifts), E[q,p] = 1 iff
1 <= p+128-q <= 3 (shifts crossing the 128-row tile boundary), and w is
the per-position preblank mask computed on-device from the int32 token
stream in one [128,17] batch. A 3-token halo tile (tile "-1") covers
runs crossing core boundaries; the halo is blank-filled at row starts,
which forces the mask to 0 there, matching the reference zero-padding.
The PSUM result is copied to SBUF on the vector engine and stored with
plain HWDGE DMAs.
"""

import numpy as np

VOCAB = 50257
DIM = 1024
BLANK = 100
B, S = 4, 4096
N_CORES = 8
TOK = B * S                  # 16384 flattened tokens
TPC = TOK // N_CORES         # 2048 tokens per core
P = 128                      # SBUF partitions
NT = TPC // P                # 16 tiles per core
HALO = 3                     # max shift distance
EXT = TPC + HALO + 1         # 2052: 3 halo + 2048 tokens + 1 pad
NMM = DIM // 512             # matmul free-dim chunks per tile

_CACHE = {}


def _shift_consts():
    """A: in-tile shift-accumulate band; E: cross-tile-boundary band; I."""
    q = np.arange(P)[:, None]
    p = np.arange(P)[None, :]
    a_mat = ((p - q >= 1) & (p - q <= HALO)).astype(np.float32)
    e_mat = ((p + P - q >= 1) & (p + P - q <= HALO)).astype(np.float32)
    i_mat = np.eye(P, dtype=np.float32)
    return a_mat, e_mat, i_mat


def _build_nc():
    from concourse import bacc, mybir, tile
    import concourse.bass as bass

    nc = bacc.Bacc(
        "TRN2", target_bir_lowering=False, debug=False, num_devices=N_CORES
    )
    i32 = mybir.dt.int32
    f32 = mybir.dt.float32
    NC = NT + 1  # tile columns incl. halo (index 0)

    ix_dram = nc.dram_tensor("ix_cols", [P, NT + 1], i32, kind="ExternalInput")
    ixn_dram = nc.dram_tensor("ixn_cols", [P, NT + 1], i32, kind="ExternalInput")
    emb = nc.dram_tensor("emb", [VOCAB, DIM], f32, kind="ExternalInput")
    a_dram = nc.dram_tensor("a_mat", [P, P], f32, kind="ExternalInput")
    e_dram = nc.dram_tensor("e_mat", [P, P], f32, kind="ExternalInput")
    i_dram = nc.dram_tensor("i_mat", [P, P], f32, kind="ExternalInput")
    out = nc.dram_tensor("out", [TPC, DIM], f32, kind="ExternalOutput")

    with tile.TileContext(nc) as tc:
        with (
            tc.tile_pool(name="sbuf", bufs=1) as pool,
            tc.tile_pool(name="psum", bufs=3, space="PSUM") as psum_pool,
        ):
            # ---- token + next-token columns (host-laid); col 0 = halo ----
            ix_all = pool.tile([P, NC], i32)
            ixn_all = pool.tile([P, NC], i32)
            nc.scalar.dma_start(out=ix_all[:], in_=ix_dram[:])
            nc.scalar.dma_start(out=ixn_all[:], in_=ixn_dram[:])
            a_sb = pool.tile([P, P], f32)
            e_sb = pool.tile([P, P], f32)
            i_sb = pool.tile([P, P], f32)
            nc.scalar.dma_start(out=a_sb[:], in_=a_dram[:])
            nc.scalar.dma_start(out=e_sb[:], in_=e_dram[:])
            nc.scalar.dma_start(out=i_sb[:], in_=i_dram[:])

            # ---- preblank masks w = isblank(next) & ~isblank(cur), batched ----
            b_all = pool.tile([P, NC], i32)
            bn_all = pool.tile([P, NC], i32)
            w_all = pool.tile([P, NC], f32)
            nc.vector.tensor_scalar(
                out=b_all[:], in0=ix_all[:], scalar1=BLANK, scalar2=None,
                op0=mybir.AluOpType.is_equal,
            )
            nc.vector.tensor_scalar(
                out=bn_all[:], in0=ixn_all[:], scalar1=BLANK, scalar2=None,
                op0=mybir.AluOpType.is_equal,
            )
            nc.vector.tensor_scalar(  # b := 1 - b
                out=b_all[:], in0=b_all[:], scalar1=-1, scalar2=1,
                op0=mybir.AluOpType.mult, op1=mybir.AluOpType.add,
            )
            nc.vector.tensor_tensor(  # bn := bn * (1 - b)
                out=bn_all[:], in0=bn_all[:], in1=b_all[:],
                op=mybir.AluOpType.mult,
            )
            nc.vector.tensor_copy(out=w_all[:], in_=bn_all[:])

            # ---- all gathers first (tile 0's rhs before the halo) so the
            # tensor engine's first matmul can start as early as possible ----
            g = [None] * NC
            for j in [1, 2, 0] + list(range(3, NC)):
                gt = pool.tile([P, DIM], f32, name=f"g{j}", tag="g", bufs=8)
                nc.gpsimd.indirect_dma_start(
                    out=gt[:], out_offset=None, in_=emb[:],
                    in_offset=bass.IndirectOffsetOnAxis(
                        ap=ix_all[:, j : j + 1], axis=0
                    ),
                )
                g[j] = gt

            # ---- per-tile matmul / copy / store chains ----
            for j in range(1, NC):
                i = j - 1  # output tile index

                m_sb = pool.tile([P, P], f32, name=f"m{i}", tag="m", bufs=6)
                ew_sb = pool.tile([P, P], f32, name=f"ew{i}", tag="ew", bufs=6)
                nc.vector.tensor_tensor(  # M = A * w_i (bcast)
                    out=m_sb[:], in0=a_sb[:],
                    in1=w_all[:, j : j + 1].to_broadcast([P, P]),
                    op=mybir.AluOpType.mult,
                )
                nc.vector.tensor_tensor(  # M += I  (fold base into matmul)
                    out=m_sb[:], in0=m_sb[:], in1=i_sb[:],
                    op=mybir.AluOpType.add,
                )
                nc.vector.tensor_tensor(  # Ew = E * w_{i-1} (bcast)
                    out=ew_sb[:], in0=e_sb[:],
                    in1=w_all[:, j - 1 : j].to_broadcast([P, P]),
                    op=mybir.AluOpType.mult,
                )

                c = psum_pool.tile([P, DIM], f32, name=f"c{i}", tag="c", bufs=4)
                for h in range(NMM):
                    sl = slice(512 * h, 512 * (h + 1))
                    nc.tensor.matmul(
                        out=c[:, sl], lhsT=m_sb[:], rhs=g[j][:, sl],
                        start=True, stop=False,
                    )
                    nc.tensor.matmul(
                        out=c[:, sl], lhsT=ew_sb[:], rhs=g[j - 1][:, sl],
                        start=False, stop=True,
                    )
                o_sb = pool.tile([P, DIM], f32, name=f"o{i}", tag="o", bufs=6)
                nc.vector.tensor_copy(out=o_sb[:], in_=c[:])
                nc.sync.dma_start(out=out[P * i : P * (i + 1), :], in_=o_sb[:])

    nc.compile()
    return nc


def get_nc():
    if "nc" not in _CACHE:
        _CACHE["nc"] = _build_nc()
    return _CACHE["nc"]


def shard_inputs(x, emb_table):
    """Build per-core in_maps from full inputs."""
    flat = np.ascontiguousarray(np.asarray(x).astype(np.int32).reshape(-1))
    emb_f32 = np.ascontiguousarray(np.asarray(emb_table, dtype=np.float32))
    a_mat, e_mat, i_mat = _shift_consts()
    in_maps = []
    for c in range(N_CORES):
        start = c * TPC
        ext = np.zeros(EXT, dtype=np.int32)
        if start % S == 0:
            # row start: blank-filled halo makes the preblank mask 0 there,
            # matching the reference's zero-padded shifts at row boundaries
            ext[:HALO] = BLANK
        else:
            ext[:HALO] = flat[start - HALO : start]
        ext[HALO : HALO + TPC] = flat[start : start + TPC]
        # ext[-1] stays 0: only read to build w at the last position, whose
        # A-matrix row is all-zero (contributions belong to the next core)
        ix_cols = np.zeros((P, NT + 1), dtype=np.int32)
        ixn_cols = np.zeros((P, NT + 1), dtype=np.int32)
        ix_cols[P - HALO :, 0] = ext[0:HALO]
        ixn_cols[P - HALO :, 0] = ext[1 : HALO + 1]
        # tile layout: token t = 128*i + p -> column i+1, partition p
        ix_cols[:, 1:] = ext[HALO : HALO + TPC].reshape(NT, P).T
        ixn_cols[:, 1:] = ext[HALO + 1 : HALO + 1 + TPC].reshape(NT, P).T
        in_maps.append(
            {"ix_cols": ix_cols, "ixn_cols": ixn_cols, "emb": emb_f32,
             "a_mat": a_mat, "e_mat": e_mat, "i_mat": i_mat}
        )
    return in_maps


def assemble_output(results):
    parts = [results[c]["out"] for c in range(N_CORES)]
    return np.concatenate(parts, axis=0).reshape(B, S, DIM)


def kernel(x, emb_table):
    from concourse.bass_utils import run_bass_kernel_spmd

    nc = get_nc()
    in_maps = shard_inputs(x, emb_table)
    res = run_bass_kernel_spmd(nc, in_maps, core_ids=list(range(N_CORES)))
    return assemble_output(res.results)
